# revision 2
# baseline (speedup 1.0000x reference)
"""2-layer GAT on 8 Trainium2 NeuronCores via Bass/Tile — v3.

Contract: kernel(**inputs) takes FULL inputs of reference.setup_inputs(),
returns the FULL [16, 4096, 128] float32 output.

v3 design (vs v1: per-chunk indirect gathers for BOTH layers, measured
~1.4us/gather SWDGE issue wall = ~3.4ms):
- Layer 1 does NO device gather at all. The host pre-gathers raw x rows per
  edge slot into two dense bf16 streams (xg = x[src], xd = x[dst]), stored
  chunk-transposed ([chunk, 128 feat, 128 edge]); the device recomputes
  h1/a_src (xg @ [W1|wsrc]) and a_dst (xd @ wdst) PER EDGE via matmul.
  Gather-to-recompute conversion: ~2.4x the L1 flops (trivial on PE) for
  zero gather instructions and full-bandwidth sequential DMA.
- dst-node sharding as before: core c owns dst blocks; per block, K1 chunks
  of 128 edges (dst-sorted). Routing matrix M via is_equal; acc += M^T@[v|w]
  in PSUM; per-block epilogue: softmax-normalize, ELU+1, @[W2|W2a2s|W2a2d]
  -> t2s row [h2' 128 | 1 | asrc2' | adst2'-c | pad] (132 cols bf16).
- Sliced AllGather of t2s into 4 per-slice shared tensors t2g0..3; layer-2
  edges are binned per (dst block, src slice) so each gather chunk depends
  on one slice only and can start as soon as that slice's AllGather lands.
- Layer 2 keeps per-chunk indirect gathers (130 cols: [h2'|1|asrc2']) — the
  irreducible cost. Per-edge a_dst select uses the max-separation identity
  exp(lrelu(u+b)) = max(exp(u)exp(b), exp(.2u)exp(.2b)): per block a
  broadcast tile A/B = exp(adst row) is built once (PE transpose + rank-1
  matmul); per chunk the full weight tile W[e,d] = max(a_e*A_d, b_e*B_d)
  is masked by M (m4w = M.*W) so the dst-select happens through the mask.
  acc[:,0:129] += m4w^T @ [h2'|1] gives both the weighted sum and sum(w).
- Softmax max-subtraction skipped (logits O(1); f32 PSUM accumulation).
- ELU -1 folded out: t2 stores h2' = (elu+1)@W2; sum(alpha)=1 folds the
  colsum(W2) correction into b2eff and the L2 logit constant c2.
"""

import os
import sys

import numpy as np
import ml_dtypes

_SKIP = set(os.environ.get("K3SKIP", "").split(","))

if "/opt/trn_rl_repo" not in sys.path:
    sys.path.insert(0, "/opt/trn_rl_repo")

import concourse.bass as bass
import concourse.bacc as bacc
import concourse.mybir as mybir
import concourse.tile as tile

F32 = mybir.dt.float32
BF16 = mybir.dt.bfloat16
I32 = mybir.dt.int32
AOP = mybir.AluOpType
ACT = mybir.ActivationFunctionType
BF = ml_dtypes.bfloat16

NEG_SLOPE = 0.2
NCORES = 8
BLK = 128
GB = 4    # blocks per load group
CG = 4    # chunks per DVE instruction group
T2W = 132  # t2 row: [0:128] h2', [128] 1.0, [129] asrc2', [130] adst2'-c2, pad
GW = 130   # gathered row width (cols 0:130)


class Cfg:
    def __init__(self, n_nodes, d_in, h1, c1, d2, k1, k2s, nslice):
        self.N = n_nodes
        self.D = d_in
        self.H1 = h1
        self.C1 = c1
        self.D1 = h1 * c1
        self.D2 = d2
        self.K1 = k1
        self.K2S = tuple(k2s)          # per-slice chunk counts
        self.K2 = sum(k2s)
        self.NSH = n_nodes // NCORES
        self.NBLK = self.NSH // BLK
        self.NGRP = self.NBLK // GB
        self.NSLICE = nslice
        self.SLN = self.NSH // nslice


# ---------------------------------------------------------------------------
# device program
# ---------------------------------------------------------------------------
def build_program(cfg):
    D, D1, D2, H1 = cfg.D, cfg.D1, cfg.D2, cfg.H1
    N, NSH, NBLK, NGRP = cfg.N, cfg.NSH, cfg.NBLK, cfg.NGRP
    K1, K2, NSLICE, SLN = cfg.K1, cfg.K2, cfg.NSLICE, cfg.SLN

    nc = bacc.Bacc("TRN2", target_bir_lowering=False, debug=False,
                   num_devices=NCORES)

    xg = nc.dram_tensor("xg", [NBLK * K1, D, 128], BF16, kind="ExternalInput")
    xd = nc.dram_tensor("xd", [NBLK * K1, D, 128], BF16, kind="ExternalInput")
    edloc1 = nc.dram_tensor("edloc1", [NGRP, 128, GB * K1], BF16,
                            kind="ExternalInput")
    esrc2 = nc.dram_tensor("esrc2", [NGRP, 128, GB * K2], I32,
                           kind="ExternalInput")
    edloc2 = nc.dram_tensor("edloc2", [NGRP, 128, GB * K2], BF16,
                            kind="ExternalInput")
    wpackS = nc.dram_tensor("wpackS", [D, D1 + H1], BF16, kind="ExternalInput")
    wdstw = nc.dram_tensor("wdstw", [D, H1], BF16, kind="ExternalInput")
    w2aug = nc.dram_tensor("w2aug", [D1, D2 + 2], BF16, kind="ExternalInput")
    b1r = nc.dram_tensor("b1r", [128, D1], F32, kind="ExternalInput")
    b2effr = nc.dram_tensor("b2effr", [128, D2], F32, kind="ExternalInput")
    iota = nc.dram_tensor("iota", [128, 128], BF16, kind="ExternalInput")
    iotac = nc.dram_tensor("iotac", [128, 1], BF16, kind="ExternalInput")
    out = nc.dram_tensor("out", [NSH, D2], F32, kind="ExternalOutput")

    t2s_kind = ("ExternalOutput" if os.environ.get("K3DUMP") == "1"
                else "Internal")
    t2s = nc.dram_tensor("t2s", [NSH, T2W], BF16, kind=t2s_kind)
    t2g = [nc.dram_tensor(f"t2g{s}", [SLN * NCORES, T2W], BF16,
                          kind="Internal", addr_space="Shared")
           for s in range(NSLICE)]

    with tile.TileContext(nc) as tc:
        with tc.tile_pool(name="const", bufs=1) as cp:
            con = {}
            for name, hndl in [("wpackS", wpackS), ("wdstw", wdstw),
                               ("w2aug", w2aug), ("b1r", b1r),
                               ("b2effr", b2effr), ("iota", iota),
                               ("iotac", iotac)]:
                t = cp.tile(list(hndl.shape), hndl.dtype, tag=name)
                nc.sync.dma_start(out=t[:], in_=hndl[:])
                con[name] = t
            ident_bf = cp.tile([128, 128], BF16)
            nc.vector.tensor_tensor(
                out=ident_bf[:], in0=con["iotac"][:].to_broadcast([128, 128]),
                in1=con["iota"][:], op=AOP.is_equal)
            con["ident_bf"] = ident_bf
            ones_row = cp.tile([1, 128], BF16)
            nc.vector.memset(ones_row[:], 1.0)
            con["ones_row"] = ones_row

            _layer1(nc, tc, cfg, xg, xd, edloc1, con, t2s, t2g)
            _layer2(nc, tc, cfg, esrc2, edloc2, con, t2s, t2g, out)

    nc.compile()
    return nc


def _layer1(nc, tc, cfg, xg, xd, edloc1, con, t2s, t2g):
    D, D1, H1 = cfg.D, cfg.D1, cfg.H1
    K1, NGRP, NBLK, NSLICE, SLN = (cfg.K1, cfg.NGRP, cfg.NBLK, cfg.NSLICE,
                                   cfg.SLN)
    RC = D1 + H1          # 72: [v 64 | w 8]
    PW = D1 + H1          # 72: ps cols [h 64 | logits 8]
    BPS = NBLK // NSLICE
    with (
        tc.tile_pool(name="l1_x", bufs=2) as px,
        tc.tile_pool(name="l1_ix", bufs=2) as pix,
        tc.tile_pool(name="l1_ps", bufs=3, space="PSUM") as pps,
        tc.tile_pool(name="l1_m", bufs=3) as pm,
        tc.tile_pool(name="l1_r", bufs=3) as pr,
        tc.tile_pool(name="l1_acc", bufs=2, space="PSUM") as pacc,
        tc.tile_pool(name="l1_ep", bufs=2) as pep,
        tc.tile_pool(name="l1_epp", bufs=1, space="PSUM") as pepp,
    ):
        for g in range(NGRP):
            XG = px.tile([D, GB * K1 * 128], BF16, tag="XG")
            nc.sync.dma_start(
                out=XG[:].rearrange("f (c e) -> f c e", e=128),
                in_=xg[g * GB * K1:(g + 1) * GB * K1].rearrange(
                    "c f e -> f c e"))
            XD = px.tile([D, GB * K1 * 128], BF16, tag="XD")
            nc.sync.dma_start(
                out=XD[:].rearrange("f (c e) -> f c e", e=128),
                in_=xd[g * GB * K1:(g + 1) * GB * K1].rearrange(
                    "c f e -> f c e"))
            dl = pix.tile([128, GB * K1], BF16, tag="dl")
            nc.sync.dma_start(out=dl[:], in_=edloc1[g])

            for i in range(GB):
                b = g * GB + i
                acc = pacc.tile([128, RC], F32, tag="acc")
                for j0 in range(0, K1, CG):
                    w = min(CG, K1 - j0)
                    ps = pps.tile([128, w * PW], F32, tag="ps")
                    for u in range(w):
                        cb = (i * K1 + j0 + u) * 128
                        nc.tensor.matmul(
                            out=ps[:, u * PW:u * PW + D1],
                            lhsT=XG[:, cb:cb + 128],
                            rhs=con["wpackS"][:, 0:D1],
                            start=True, stop=True)
                        # logits: asrc then adst accumulated into same cols
                        nc.tensor.matmul(
                            out=ps[:, u * PW + D1:u * PW + D1 + H1],
                            lhsT=XG[:, cb:cb + 128],
                            rhs=con["wpackS"][:, D1:D1 + H1],
                            start=True, stop=False)
                        nc.tensor.matmul(
                            out=ps[:, u * PW + D1:u * PW + D1 + H1],
                            lhsT=XD[:, cb:cb + 128], rhs=con["wdstw"][:],
                            start=False, stop=True)
                    e1 = pr.tile([128, w * H1], F32, tag="e1")
                    nc.scalar.activation(
                        out=e1[:].rearrange("p (c h) -> p c h", c=w),
                        in_=_sl3(ps[:], PW, w, H1, D1), func=ACT.Exp)
                    ls = pr.tile([128, w * H1], F32, tag="ls")
                    nc.vector.tensor_scalar_mul(
                        out=ls[:].rearrange("p (c h) -> p c h", c=w),
                        in0=_sl3(ps[:], PW, w, H1, D1), scalar1=NEG_SLOPE)
                    e2 = pr.tile([128, w * H1], F32, tag="e2")
                    nc.scalar.activation(out=e2[:], in_=ls[:], func=ACT.Exp)
                    # w in bf16 once; reuse the SAME rounded w for v=h*w and
                    # the sum channel so its rounding cancels in the softmax
                    rhs = pr.tile([128, w * RC], BF16, tag="rhs")
                    nc.vector.tensor_tensor(
                        out=_sl3(rhs[:], RC, w, H1, D1),
                        in0=e1[:].rearrange("p (c h) -> p c h", c=w),
                        in1=e2[:].rearrange("p (c h) -> p c h", c=w),
                        op=AOP.max)
                    wf = pr.tile([128, w * H1], F32, tag="wf")
                    nc.vector.tensor_copy(
                        out=wf[:].rearrange("p (c h) -> p c h", c=w),
                        in_=_sl3(rhs[:], RC, w, H1, D1))
                    nc.vector.tensor_tensor(
                        out=_s4(rhs[:], RC, w, H1, D1 // H1, 0),
                        in0=_s4(ps[:], PW, w, H1, D1 // H1, 0),
                        in1=_b4(wf[:], H1, w, D1 // H1),
                        op=AOP.mult)
                    m4 = pm.tile([128, w * 128], BF16, tag="m4")
                    nc.vector.tensor_tensor(
                        out=m4[:].rearrange("p (c d) -> p c d", c=w),
                        in0=_bl(dl[:, i * K1 + j0:i * K1 + j0 + w], 128),
                        in1=_b3(con["iota"][:], w),
                        op=AOP.is_equal)
                    for u in range(w):
                        nc.tensor.matmul(
                            out=acc[:], lhsT=m4[:, u * 128:(u + 1) * 128],
                            rhs=rhs[:, u * RC:(u + 1) * RC],
                            start=(j0 == 0 and u == 0),
                            stop=(j0 + w == K1 and u == w - 1))

                _epi1(nc, cfg, con, acc, t2s, b, pep, pepp)
                if (b + 1) % BPS == 0 and "ag" not in _SKIP:
                    s = (b + 1) // BPS - 1
                    nc.gpsimd.collective_compute(
                        "AllGather", AOP.bypass,
                        replica_groups=[list(range(NCORES))],
                        ins=[t2s[s * SLN:(s + 1) * SLN, :]],
                        outs=[t2g[s][:]])


def _epi1(nc, cfg, con, acc, t2s, b, pep, pepp):
    H1, C1, D1, D2 = cfg.H1, cfg.C1, cfg.D1, cfg.D2
    accs = pep.tile([128, D1 + H1], F32, tag="accs")
    nc.vector.tensor_copy(out=accs[:], in_=acc[:])
    sinv = pep.tile([128, H1], F32, tag="sinv")
    nc.vector.reciprocal(out=sinv[:], in_=accs[:, D1:D1 + H1])
    y = pep.tile([128, D1], F32, tag="y")
    nc.vector.tensor_tensor(
        out=y[:].rearrange("p (h c) -> p h c", h=H1),
        in0=accs[:, 0:D1].rearrange("p (h c) -> p h c", h=H1),
        in1=_bc_hc(sinv[:], H1, C1), op=AOP.mult)
    nc.vector.tensor_add(out=y[:], in0=y[:], in1=con["b1r"][:])
    tmin = pep.tile([128, D1], F32, tag="tmin")
    nc.vector.tensor_scalar_min(out=tmin[:], in0=y[:], scalar1=0.0)
    e_t = pep.tile([128, D1], F32, tag="e")
    nc.scalar.activation(out=e_t[:], in_=tmin[:], func=ACT.Exp)
    em1 = pep.tile([128, D1], F32, tag="em1")
    nc.vector.tensor_scalar_add(out=em1[:], in0=e_t[:], scalar1=-1.0)
    helu = pep.tile([128, D1], BF16, tag="helu")
    nc.vector.scalar_tensor_tensor(
        out=helu[:], in0=y[:], scalar=0.0, in1=em1[:], op0=AOP.max,
        op1=AOP.add)
    htp = pepp.tile([D1, 128], BF16, tag="htp")
    nc.tensor.transpose(out=htp[:], in_=helu[:], identity=con["ident_bf"][:])
    hts = pep.tile([D1, 128], BF16, tag="hts")
    nc.vector.tensor_copy(out=hts[:], in_=htp[:])
    h2p = pepp.tile([128, D2 + 2], F32, tag="h2p")
    nc.tensor.matmul(out=h2p[:], lhsT=hts[:], rhs=con["w2aug"][:],
                     start=True, stop=True)
    stg2 = pep.tile([128, T2W], BF16, tag="stg2")
    nc.vector.tensor_copy(out=stg2[:, 0:D2], in_=h2p[:, 0:D2])
    nc.vector.memset(stg2[:, D2:D2 + 1], 1.0)
    nc.vector.tensor_copy(out=stg2[:, D2 + 1:D2 + 3], in_=h2p[:, D2:D2 + 2])
    nc.vector.memset(stg2[:, D2 + 3:T2W], 0.0)
    nc.sync.dma_start(out=t2s[b * BLK:(b + 1) * BLK, :], in_=stg2[:])


def _layer2(nc, tc, cfg, esrc2, edloc2, con, t2s, t2g, out):
    D2 = cfg.D2
    K2, K2S, NGRP, NSLICE = cfg.K2, cfg.K2S, cfg.NGRP, cfg.NSLICE
    RC = D2 + 1
    # chunk index -> slice
    sl_of = []
    for s, k in enumerate(K2S):
        sl_of += [s] * k
    # chunk groups that don't straddle slices
    spans = []
    base = 0
    for s, k in enumerate(K2S):
        for j0 in range(0, k, CG):
            spans.append((base + j0, min(CG, k - j0), s))
        base += k
    if "l2" in _SKIP:
        with tc.tile_pool(name="l2z", bufs=2) as pz:
            for b in range(cfg.NBLK):
                o2 = pz.tile([128, D2], F32, tag="o2")
                nc.vector.memset(o2[:], 0.0)
                nc.sync.dma_start(out=out[b * BLK:(b + 1) * BLK, :], in_=o2[:])
        return
    with (
        tc.tile_pool(name="l2_ix", bufs=2) as pix,
        tc.tile_pool(name="l2_g", bufs=4) as pg,
        tc.tile_pool(name="l2_m", bufs=3) as pm,
        tc.tile_pool(name="l2_r", bufs=3) as pr,
        tc.tile_pool(name="l2_ab", bufs=2) as pab,
        tc.tile_pool(name="l2_abp", bufs=1, space="PSUM") as pabp,
        tc.tile_pool(name="l2_acc", bufs=2, space="PSUM") as pacc,
        tc.tile_pool(name="l2_ep", bufs=2) as pep,
    ):
        for g in range(NGRP):
            ix = pix.tile([128, GB * K2], I32, tag="ix")
            nc.sync.dma_start(out=ix[:], in_=esrc2[g])
            dl = pix.tile([128, GB * K2], BF16, tag="dl")
            nc.sync.dma_start(out=dl[:], in_=edloc2[g])
            adw = pix.tile([128, GB], BF16, tag="adw")
            nc.sync.dma_start(
                out=adw[:].rearrange("p (b o) -> p b o", o=1),
                in_=bass.AP(t2s[:].tensor, (g * GB * 128) * T2W + D2 + 2,
                            [[T2W, 128], [128 * T2W, GB], [1, 1]]))
            # per-group A/B broadcast prep: exp(adw), exp(.2 adw) -> rows
            adws = pix.tile([128, GB], BF16, tag="adws")
            nc.vector.tensor_scalar_mul(out=adws[:], in0=adw[:],
                                        scalar1=NEG_SLOPE)
            eab = pab.tile([128, 2 * GB], BF16, tag="eab")
            nc.scalar.activation(out=eab[:, 0:GB], in_=adw[:], func=ACT.Exp)
            nc.scalar.activation(out=eab[:, GB:2 * GB], in_=adws[:],
                                 func=ACT.Exp)
            abbc = pab.tile([128, 2 * GB * 128], BF16, tag="abbc")
            for i in range(2 * GB):
                rowp = pabp.tile([1, 128], BF16, tag="rowp")
                nc.tensor.transpose(out=rowp[:], in_=eab[:, i:i + 1],
                                    identity=con["ident_bf"][:])
                row = pab.tile([1, 128], BF16, tag="row")
                nc.vector.tensor_copy(out=row[:], in_=rowp[:])
                bcp = pabp.tile([128, 128], F32, tag="bcp")
                nc.tensor.matmul(out=bcp[:], lhsT=con["ones_row"][:],
                                 rhs=row[:], start=True, stop=True)
                nc.vector.tensor_copy(out=abbc[:, i * 128:(i + 1) * 128],
                                      in_=bcp[:])

            for i in range(GB):
                b = g * GB + i
                Abc = abbc[:, i * 128:(i + 1) * 128]
                Bbc = abbc[:, (GB + i) * 128:(GB + i + 1) * 128]
                acc = pacc.tile([128, RC], F32, tag="acc")
                for (j0, w, s) in spans:
                    first = j0 == 0
                    last = j0 + w == K2
                    gt = pg.tile([128, w * GW], BF16, tag="gt")
                    if "gather" in _SKIP:
                        nc.vector.memset(gt[:], 1.0)
                    else:
                        for u in range(w):
                            nc.gpsimd.indirect_dma_start(
                                out=gt[:, u * GW:(u + 1) * GW],
                                out_offset=None, in_=t2g[s][:],
                                in_offset=bass.IndirectOffsetOnAxis(
                                    ap=ix[:, i * K2 + j0 + u:
                                          i * K2 + j0 + u + 1],
                                    axis=0))
                    m4 = pm.tile([128, w * 128], BF16, tag="m4")
                    nc.vector.tensor_tensor(
                        out=m4[:].rearrange("p (c d) -> p c d", c=w),
                        in0=_bl(dl[:, i * K2 + j0:i * K2 + j0 + w], 128),
                        in1=_b3(con["iota"][:], w),
                        op=AOP.is_equal)
                    asl = pr.tile([128, w], BF16, tag="asl")
                    nc.vector.tensor_scalar_mul(
                        out=asl[:].rearrange("p (c o) -> p c o", o=1),
                        in0=_sl3(gt[:], GW, w, 1, GW - 1), scalar1=NEG_SLOPE)
                    ab2 = pr.tile([128, 2 * w], BF16, tag="ab2")
                    nc.scalar.activation(
                        out=ab2[:, 0:w].rearrange("p (c o) -> p c o", o=1),
                        in_=_sl3(gt[:], GW, w, 1, GW - 1), func=ACT.Exp)
                    nc.scalar.activation(
                        out=ab2[:, w:2 * w], in_=asl[:], func=ACT.Exp)
                    ta = pm.tile([128, w * 128], BF16, tag="ta")
                    nc.vector.tensor_tensor(
                        out=ta[:].rearrange("p (c d) -> p c d", c=w),
                        in0=_bl(ab2[:, 0:w], 128), in1=_b3(Abc, w),
                        op=AOP.mult)
                    tb = pm.tile([128, w * 128], BF16, tag="tb")
                    nc.vector.tensor_tensor(
                        out=tb[:].rearrange("p (c d) -> p c d", c=w),
                        in0=_bl(ab2[:, w:2 * w], 128), in1=_b3(Bbc, w),
                        op=AOP.mult)
                    nc.vector.tensor_tensor(out=ta[:], in0=ta[:], in1=tb[:],
                                            op=AOP.max)
                    m4w = pm.tile([128, w * 128], BF16, tag="m4w")
                    nc.vector.tensor_tensor(out=m4w[:], in0=m4[:], in1=ta[:],
                                            op=AOP.mult)
                    for u in range(w):
                        nc.tensor.matmul(
                            out=acc[:], lhsT=m4w[:, u * 128:(u + 1) * 128],
                            rhs=gt[:, u * GW:u * GW + RC],
                            start=(first and u == 0), stop=(last and u == w - 1))

                sinv = pep.tile([128, 1], F32, tag="sinv")
                nc.vector.reciprocal(out=sinv[:], in_=acc[:, D2:D2 + 1])
                o1 = pep.tile([128, D2], F32, tag="o1")
                nc.vector.tensor_tensor(
                    out=o1[:], in0=acc[:, 0:D2],
                    in1=sinv[:].to_broadcast([128, D2]), op=AOP.mult)
                o2 = pep.tile([128, D2], F32, tag="o2")
                nc.vector.tensor_add(out=o2[:], in0=o1[:], in1=con["b2effr"][:])
                nc.sync.dma_start(out=out[b * BLK:(b + 1) * BLK, :], in_=o2[:])


# ---- AP helpers -----------------------------------------------------------
def _sl3(ap, cstride, w, width, off):
    """[128, w, width] view: [[p], [cstride, w], [1, width]] at col offset."""
    a = [list(p) for p in ap.ap]
    return bass.AP(ap.tensor, ap.offset + off, [a[0], [cstride, w], [1, width]])


def _s4(ap, cstride, w, h, c, off):
    a = [list(p) for p in ap.ap]
    return bass.AP(ap.tensor, ap.offset + off,
                   [a[0], [cstride, w], [c, h], [1, c]])


def _b4(ap, h, w, c):
    """[128, w*h] -> [128, w, h, c] with 0-stride c."""
    a = [list(p) for p in ap.ap]
    return bass.AP(ap.tensor, ap.offset, [a[0], [h, w], [1, h], [0, c]])


def _b3(ap, g):
    a = [list(p) for p in ap.ap]
    return bass.AP(ap.tensor, ap.offset, [a[0], [0, g], a[1]])


def _bl(ap, n):
    """[128, w] slice -> [128, w, n] with 0-stride last dim."""
    a = [list(p) for p in ap.ap]
    return bass.AP(ap.tensor, ap.offset, [a[0], a[1], [0, n]])


def _bc_hc(w_ap, h, c):
    a = [list(p) for p in w_ap.ap]
    return bass.AP(w_ap.tensor, w_ap.offset, [a[0], a[1], [0, c]])


# ---------------------------------------------------------------------------
# host glue
# ---------------------------------------------------------------------------
def _cdiv(a, b):
    return -(-a // b)


def prepare(x, seq, edges, W1, att_src1, att_dst1, b1, W2, att_src2,
            att_dst2, b2, nslice=4):
    nb, ncn, d = x.shape
    N = nb * ncn
    H1, C1 = att_src1.shape
    D1 = H1 * C1
    D2 = W2.shape[1]
    NSH = N // NCORES
    nblk = NSH // BLK
    ngrp = nblk // GB
    SLN = NSH // nslice

    xf = (np.asarray(x, np.float32).reshape(N, d)
          * np.asarray(seq, np.float32).reshape(N, 1))
    src = np.concatenate([np.asarray(edges[0], np.int64),
                          np.arange(N, dtype=np.int64)])
    dst = np.concatenate([np.asarray(edges[1], np.int64),
                          np.arange(N, dtype=np.int64)])
    order = np.argsort(dst, kind="stable")
    src = src[order]
    dst = dst[order]
    blk_of = dst // BLK
    nblk_g = N // BLK
    counts = np.bincount(blk_of, minlength=nblk_g)
    starts = np.zeros(nblk_g + 1, dtype=np.int64)
    np.cumsum(counts, out=starts[1:])

    # layer-2 phys mapping: slice s of node = (loc // SLN); row within the
    # per-slice gathered tensor = rank*SLN + loc%SLN
    rank = src // NSH
    loc = src % NSH
    sl2 = loc // SLN
    row2 = rank * SLN + (loc % SLN)

    k1 = int(max(_cdiv(int(counts.max()), 128), 1))
    # per-(block, slice) counts for K2S
    k2s = []
    for s in range(nslice):
        m = 0
        for gblk in range(nblk_g):
            s0, s1 = int(starts[gblk]), int(starts[gblk + 1])
            m = max(m, int((sl2[s0:s1] == s).sum()))
        k2s.append(max(_cdiv(m, 128), 1))
    k2 = sum(k2s)

    cfg = Cfg(N, d, H1, C1, D2, k1, k2s, nslice)

    w1 = np.asarray(W1, np.float32)
    wsrc = np.einsum("khc,hc->kh", w1.reshape(d, H1, C1),
                     np.asarray(att_src1, np.float32))
    wdst = np.einsum("khc,hc->kh", w1.reshape(d, H1, C1),
                     np.asarray(att_dst1, np.float32))
    wpackS = np.concatenate([w1, wsrc], axis=1).astype(BF)
    wdstw = wdst.astype(BF)

    w2a = np.asarray(W2, np.float32)
    a2s = np.asarray(att_src2, np.float32).reshape(-1)
    a2d = np.asarray(att_dst2, np.float32).reshape(-1)
    c2_const = 0.0
    cfg.c2_const = c2_const
    b2eff = np.asarray(b2, np.float32)
    w2aug = np.concatenate([w2a, (w2a @ a2s)[:, None], (w2a @ a2d)[:, None]],
                           axis=1).astype(BF)

    b1r = np.tile(np.asarray(b1, np.float32)[None, :], (128, 1))
    b2effr = np.tile(b2eff[None, :], (128, 1))
    iota = np.tile(np.arange(128, dtype=np.float32)[None, :],
                   (128, 1)).astype(BF)
    iotac = np.arange(128, dtype=np.float32)[:, None].astype(BF)

    xf_bf = xf.astype(BF)
    in_maps = []
    for c in range(NCORES):
        # per-block slot assembly
        sA = np.zeros((nblk, k1 * 128), dtype=np.int64)      # L1 src node
        dA = np.zeros((nblk, k1 * 128), dtype=np.int64)      # L1 dst node
        vA = np.full((nblk, k1 * 128), -1.0, dtype=np.float32)
        i2 = np.zeros((nblk, k2 * 128), dtype=np.int64)      # L2 row in slice
        v2 = np.full((nblk, k2 * 128), -1.0, dtype=np.float32)
        for bb in range(nblk):
            gblk = c * nblk + bb
            s0, s1 = int(starts[gblk]), int(starts[gblk + 1])
            es = src[s0:s1]
            ed = dst[s0:s1]
            dloc = (ed - gblk * BLK).astype(np.float32)
            n = s1 - s0
            sA[bb, :n] = es
            dA[bb, :n] = ed
            vA[bb, :n] = dloc
            # L2: bin by src slice
            base = 0
            for s in range(nslice):
                selp = sl2[s0:s1] == s
                ns = int(selp.sum())
                i2[bb, base:base + ns] = row2[s0:s1][selp]
                v2[bb, base:base + ns] = dloc[selp]
                base += k2s[s] * 128
        # L1 streams: [nblk*k1, 128 feat, 128 edge] — slot (b,j,p) = chunk
        # position j*128+p; zero rows for pad slots.
        xg_c = np.zeros((nblk * k1 * 128, d), dtype=BF)
        real = vA.reshape(-1) >= 0
        xg_c[real] = xf_bf[sA.reshape(-1)[real]]
        xg_c = np.ascontiguousarray(
            xg_c.reshape(nblk * k1, 128, d).transpose(0, 2, 1))
        xd_c = np.zeros((nblk * k1 * 128, d), dtype=BF)
        xd_c[real] = xf_bf[dA.reshape(-1)[real]]
        xd_c = np.ascontiguousarray(
            xd_c.reshape(nblk * k1, 128, d).transpose(0, 2, 1))
        # edloc tiles: [ngrp, 128, GB*k]: col i*k+j = dloc of (p, chunk j)
        dl1 = vA.reshape(ngrp, GB, k1, 128).transpose(0, 3, 1, 2).reshape(
            ngrp, 128, GB * k1).astype(BF)
        dl2 = v2.reshape(ngrp, GB, k2, 128).transpose(0, 3, 1, 2).reshape(
            ngrp, 128, GB * k2).astype(BF)
        ix2 = i2.reshape(ngrp, GB, k2, 128).transpose(0, 3, 1, 2).reshape(
            ngrp, 128, GB * k2).astype(np.int32)
        in_maps.append({
            "xg": xg_c, "xd": xd_c, "edloc1": dl1, "esrc2": ix2,
            "edloc2": dl2, "wpackS": wpackS, "wdstw": wdstw, "w2aug": w2aug,
            "b1r": b1r, "b2effr": b2effr, "iota": iota, "iotac": iotac,
        })
    return cfg, c2_const, in_maps


_CACHE = {}
LAST_RESULT = None


def kernel(**inputs) -> np.ndarray:
    from concourse.bass_utils import run_bass_kernel_spmd

    global LAST_RESULT
    x = np.asarray(inputs["x"])
    nb, ncn, d = x.shape
    cfg, c2_const, in_maps = prepare(**{k: inputs[k] for k in (
        "x", "seq", "edges", "W1", "att_src1", "att_dst1", "b1",
        "W2", "att_src2", "att_dst2", "b2")})

    key = (cfg.N, cfg.D, cfg.H1, cfg.C1, cfg.D2, cfg.K1, cfg.K2S,
           cfg.NSLICE, round(c2_const, 10))
    if key not in _CACHE:
        _CACHE.clear()
        _CACHE[key] = build_program(cfg)
    nc = _CACHE[key]

    res = run_bass_kernel_spmd(nc, in_maps, core_ids=list(range(NCORES)),
                               trace=False)
    LAST_RESULT = res
    shards = [res.results[c]["out"] for c in range(NCORES)]
    full = np.concatenate(shards, axis=0)
    return full.reshape(nb, ncn, d).astype(np.float32)
